# revision 45
# baseline (speedup 1.0000x reference)
"""Bass/Tile kernel for nn_DotAttention (batched dot-product attention).

  scores[b, t] = <hidden_decoder[b], hiddens_encoder[b, t]>
  a = softmax(scores, axis=t)
  context[b, f] = sum_t a[b, t] * hiddens_encoder[b, t, f]

Full shapes: hidden_decoder (64, 1024) f32, hiddens_encoder (64, 2048, 1024) f32,
output (64, 1024) f32.

Sharding: data-parallel over batch across 8 NeuronCores (8 batches/core),
no cross-device communication.

Per-core strategy (memory-bound target -- hiddens_encoder is read from HBM
exactly once):
  - he[b] loaded in natural layout (t on partitions) via contiguous 1 MiB DMAs.
  - scores: fused multiply+rowsum on VectorE (tensor_tensor_reduce) against a
    broadcast copy of hidden_decoder[b] (PE ones-matmul broadcast).
  - softmax: DVE free-dim max, GpSimd cross-partition max, ACT exp with
    per-partition -max bias.
  - context: PE accumulating matmuls (contraction over t = partition dim),
    with an extra ones-column matmul producing Z = sum(exp) for free.
  - normalize by 1/Z during the PSUM->SBUF copy on ScalarE.
"""

import numpy as np

import concourse.bacc as bacc
import concourse.tile as tile
from concourse import mybir
from concourse.bass_utils import run_bass_kernel_spmd

N_CORES = 8
B_FULL = 64
B = B_FULL // N_CORES  # batches per core
T = 2048
F = 1024
P = 128
NT = T // P  # 16 t-chunks of 128
NCC = NT // 2  # 8 DMA loads of 2 chunks (1 MiB each)

F32 = mybir.dt.float32
F16 = mybir.dt.float16

_cache = {}


def _build():
    nc = bacc.Bacc("TRN2", target_bir_lowering=False, debug=False, num_devices=N_CORES)
    he = nc.dram_tensor("he", [B, T, F], F32, kind="ExternalInput").ap()
    hd = nc.dram_tensor("hd", [1, B * F], F32, kind="ExternalInput").ap()
    ident_d = nc.dram_tensor("ident", [P, P], F32, kind="ExternalInput").ap()
    out = nc.dram_tensor("out", [B, F], F32, kind="ExternalOutput").ap()
    zout = nc.dram_tensor("z", [1, B], F32, kind="ExternalOutput").ap()

    with tile.TileContext(nc) as tc:
        with (
            tc.tile_pool(name="consts", bufs=1) as consts,
            tc.tile_pool(name="hepool", bufs=10) as hepool,
            tc.tile_pool(name="he16pool", bufs=12) as he16pool,
            tc.tile_pool(name="hbc", bufs=B) as hbc,
            tc.tile_pool(name="small", bufs=3) as small,
            tc.tile_pool(name="outp", bufs=3) as outp,
            tc.tile_pool(name="psum", bufs=2, space="PSUM") as psum_pool,
            tc.tile_pool(name="psbc", bufs=2, space="PSUM") as psbc_pool,
        ):
            neg_ones_row = consts.tile([1, P], F32)  # lhsT for -max broadcast
            nc.vector.memset(neg_ones_row[:], -1.0)
            ones_col = consts.tile([P, 1], F16)  # rhs column for Z accumulation
            nc.vector.memset(ones_col[:], 1.0)
            ident = consts.tile([P, P], F32)  # identity for PE transpose
            nc.sync.dma_start(out=ident[:], in_=ident_d[:])

            ones_row = consts.tile([1, P], F32)
            nc.vector.memset(ones_row[:], 1.0)

            # broadcast hd[b] to all 128 partitions: ones(1,128)^T @ hd_row(1,F)
            hdb = []
            for b in range(B):
                hd_row = small.tile([1, F], F32, tag="hdrow")
                nc.sync.dma_start(out=hd_row[:], in_=hd[0:1, b * F : (b + 1) * F])
                t_b = hbc.tile([P, F], F32)
                for j in range(2):
                    ps = psbc_pool.tile([P, 512], F32, tag="misc")
                    nc.tensor.matmul(
                        ps[:],
                        lhsT=ones_row[:],
                        rhs=hd_row[0:1, j * 512 : (j + 1) * 512],
                        start=True,
                        stop=True,
                    )
                    nc.scalar.copy(t_b[:, j * 512 : (j + 1) * 512], ps[:])
                hdb.append(t_b)

            zrow = consts.tile([1, B], F32)

            for b in range(B):
                S = small.tile([P, NT], F32)
                dummy = small.tile([P, 1], F32)
                hets = []
                for cc in range(NCC):
                    het = hepool.tile([P, 2, F], F32)
                    dma_eng = nc.gpsimd if cc % 2 == 0 else nc.sync
                    dma_eng.dma_start(
                        out=het[:],
                        in_=he[b, cc * 256 : (cc + 1) * 256, :].rearrange(
                            "(c p) f -> p c f", p=P
                        ),
                    )
                    # fp16 copy for the context matmul (ACT)
                    het16 = he16pool.tile([P, 2, F], F16)
                    nc.scalar.copy(het16[:], het[:])
                    hets.append(het16)
                    # tiny PE op paced by the cast stream: keeps the HAM
                    # activity window fed so ctx matmuls run at 2.4 GHz
                    psw = psbc_pool.tile([1, 1], F32, tag="misc")
                    nc.tensor.matmul(
                        psw[:],
                        lhsT=ones_col[:],
                        rhs=het16[:, 0, 0:1],
                        start=True,
                        stop=True,
                    )
                    for c in range(2):
                        col = cc * 2 + c
                        nc.vector.scalar_tensor_tensor(
                            dummy.broadcast_to((P, F)),
                            het[:, c, :],
                            1.0,
                            hdb[b][:],
                            op0=mybir.AluOpType.mult,
                            op1=mybir.AluOpType.mult,
                            accum_out=S[:, col : col + 1],
                        )

                # softmax over all T = 128 partitions x NT columns
                m1 = small.tile([P, 1], F32)
                nc.vector.reduce_max(m1[:], S[:], axis=mybir.AxisListType.X)
                # cross-partition max: PE transpose -> DVE reduce -> -1s matmul
                pst = psbc_pool.tile([1, P], F32, tag="misc")
                nc.tensor.transpose(pst[:], m1[:], ident[:])
                M_sb = small.tile([1, 1], F32)
                nc.vector.reduce_max(M_sb[:], pst[:], axis=mybir.AxisListType.X)
                psb = psbc_pool.tile([P, 1], F32, tag="misc")
                nc.tensor.matmul(
                    psb[:], lhsT=neg_ones_row[:], rhs=M_sb[:], start=True, stop=True
                )
                negm = small.tile([P, 1], F32)
                nc.scalar.copy(negm[:], psb[:])
                E = small.tile([P, NT], F32)
                nc.scalar.activation(
                    E[:],
                    S[:],
                    mybir.ActivationFunctionType.Exp,
                    bias=negm[:],
                    scale=1.0,
                )

                E16 = small.tile([P, NT], F16)
                nc.scalar.copy(E16[:], E[:])

                # context = sum_t E[t] * he[t, :]; Z = sum_t E[t]
                psA = psum_pool.tile([1, 512], F32)
                psB = psum_pool.tile([1, 512], F32)
                psZ = psum_pool.tile([1, 1], F32)
                for col in range(NT):
                    chunk = hets[col // 2][:, col % 2, :]
                    st = col == 0
                    sp = col == NT - 1
                    w = E16[:, col : col + 1]
                    nc.tensor.matmul(psA[:], lhsT=w, rhs=chunk[:, 0:512], start=st, stop=sp)
                    nc.tensor.matmul(psB[:], lhsT=w, rhs=chunk[:, 512:1024], start=st, stop=sp)
                    nc.tensor.matmul(psZ[:], lhsT=w, rhs=ones_col[:], start=st, stop=sp)

                # unnormalized context + Z out; host divides (keeps DVE stream
                # free of cross-engine waits)
                ob = outp.tile([1, F], F32)
                nc.scalar.copy(ob[0:1, 0:512], psA[:])
                nc.scalar.copy(ob[0:1, 512:1024], psB[:])
                nc.scalar.copy(zrow[0:1, b : b + 1], psZ[:])
                nc.sync.dma_start(out=out[b : b + 1, :], in_=ob[:])
            nc.sync.dma_start(out=zout[:], in_=zrow[:])

    nc.compile()
    return nc


def _get_nc():
    if "nc" not in _cache:
        _cache["nc"] = _build()
    return _cache["nc"]


def _run(hidden_decoder, hiddens_encoder, trace=False, tmpdir=None):
    nc = _get_nc()
    hidden_decoder = np.ascontiguousarray(hidden_decoder, dtype=np.float32)
    hiddens_encoder = np.ascontiguousarray(hiddens_encoder, dtype=np.float32)
    ident = np.eye(P, dtype=np.float32)
    in_maps = [
        {
            "he": hiddens_encoder[i * B : (i + 1) * B],
            "hd": hidden_decoder[i * B : (i + 1) * B].reshape(1, B * F),
            "ident": ident,
        }
        for i in range(N_CORES)
    ]
    res = run_bass_kernel_spmd(
        nc, in_maps, list(range(N_CORES)), trace=trace, tmpdir=tmpdir
    )
    out = np.concatenate(
        [
            res.results[i]["out"] / res.results[i]["z"].reshape(B, 1)
            for i in range(N_CORES)
        ],
        axis=0,
    ).astype(np.float32)
    return out, res


def kernel(hidden_decoder, hiddens_encoder):
    out, _ = _run(hidden_decoder, hiddens_encoder)
    return out


# revision 47
# speedup vs baseline: 1.0058x; 1.0058x over previous
"""Bass/Tile kernel for nn_DotAttention (batched dot-product attention).

  scores[b, t] = <hidden_decoder[b], hiddens_encoder[b, t]>
  a = softmax(scores, axis=t)
  context[b, f] = sum_t a[b, t] * hiddens_encoder[b, t, f]

Full shapes: hidden_decoder (64, 1024) f32, hiddens_encoder (64, 2048, 1024) f32,
output (64, 1024) f32.

Sharding: data-parallel over batch across 8 NeuronCores (8 batches/core),
no cross-device communication.

Per-core strategy (memory-bound target -- hiddens_encoder is read from HBM
exactly once):
  - he[b] loaded in natural layout (t on partitions) via contiguous 1 MiB DMAs.
  - scores: fused multiply+rowsum on VectorE (tensor_tensor_reduce) against a
    broadcast copy of hidden_decoder[b] (PE ones-matmul broadcast).
  - softmax: DVE free-dim max, GpSimd cross-partition max, ACT exp with
    per-partition -max bias.
  - context: PE accumulating matmuls (contraction over t = partition dim),
    with an extra ones-column matmul producing Z = sum(exp) for free.
  - normalize by 1/Z during the PSUM->SBUF copy on ScalarE.
"""

import numpy as np

import concourse.bacc as bacc
import concourse.tile as tile
from concourse import mybir
from concourse.bass_utils import run_bass_kernel_spmd

N_CORES = 8
B_FULL = 64
B = B_FULL // N_CORES  # batches per core
T = 2048
F = 1024
P = 128
NT = T // P  # 16 t-chunks of 128
NCC = NT // 2  # 8 DMA loads of 2 chunks (1 MiB each)

F32 = mybir.dt.float32
F16 = mybir.dt.float16

_cache = {}


def _build():
    nc = bacc.Bacc("TRN2", target_bir_lowering=False, debug=False, num_devices=N_CORES)
    he = nc.dram_tensor("he", [B, T, F], F32, kind="ExternalInput").ap()
    hd = nc.dram_tensor("hd", [1, B * F], F32, kind="ExternalInput").ap()
    ident_d = nc.dram_tensor("ident", [P, P], F32, kind="ExternalInput").ap()
    out = nc.dram_tensor("out", [B, F], F32, kind="ExternalOutput").ap()
    zout = nc.dram_tensor("z", [1, B], F32, kind="ExternalOutput").ap()

    with tile.TileContext(nc) as tc:
        with (
            tc.tile_pool(name="consts", bufs=1) as consts,
            tc.tile_pool(name="hepool", bufs=10) as hepool,
            tc.tile_pool(name="he16pool", bufs=12) as he16pool,
            tc.tile_pool(name="hbc", bufs=B) as hbc,
            tc.tile_pool(name="small", bufs=3) as small,
            tc.tile_pool(name="outp", bufs=3) as outp,
            tc.tile_pool(name="psum", bufs=2, space="PSUM") as psum_pool,
            tc.tile_pool(name="psbc", bufs=2, space="PSUM") as psbc_pool,
        ):
            neg_ones_row = consts.tile([1, P], F32)  # lhsT for -max broadcast
            nc.vector.memset(neg_ones_row[:], -1.0)
            ones_col = consts.tile([P, 1], F16)  # rhs column for Z accumulation
            nc.vector.memset(ones_col[:], 1.0)
            ident = consts.tile([P, P], F32)  # identity for PE transpose
            nc.sync.dma_start(out=ident[:], in_=ident_d[:])

            ones_row = consts.tile([1, P], F32)
            nc.vector.memset(ones_row[:], 1.0)

            # broadcast hd[b] to all 128 partitions: ones(1,128)^T @ hd_row(1,F)
            hdb = []
            for b in range(B):
                hd_row = small.tile([1, F], F32, tag="hdrow")
                nc.sync.dma_start(out=hd_row[:], in_=hd[0:1, b * F : (b + 1) * F])
                t_b = hbc.tile([P, F], F32)
                for j in range(2):
                    ps = psbc_pool.tile([P, 512], F32, tag="misc")
                    nc.tensor.matmul(
                        ps[:],
                        lhsT=ones_row[:],
                        rhs=hd_row[0:1, j * 512 : (j + 1) * 512],
                        start=True,
                        stop=True,
                    )
                    nc.scalar.copy(t_b[:, j * 512 : (j + 1) * 512], ps[:])
                hdb.append(t_b)

            zrow = consts.tile([1, B], F32)

            for b in range(B):
                S = small.tile([P, NT], F32)
                dummy = small.tile([P, 1], F32)
                hets = []
                for cc in range(NCC):
                    het = hepool.tile([P, 2, F], F32)
                    nc.gpsimd.dma_start(
                        out=het[:],
                        in_=he[b, cc * 256 : (cc + 1) * 256, :].rearrange(
                            "(c p) f -> p c f", p=P
                        ),
                    )
                    # fp16 copy for the context matmul (ACT)
                    het16 = he16pool.tile([P, 2, F], F16)
                    nc.scalar.copy(het16[:], het[:])
                    hets.append(het16)
                    for c in range(2):
                        col = cc * 2 + c
                        nc.vector.scalar_tensor_tensor(
                            dummy.broadcast_to((P, F)),
                            het[:, c, :],
                            1.0,
                            hdb[b][:],
                            op0=mybir.AluOpType.mult,
                            op1=mybir.AluOpType.mult,
                            accum_out=S[:, col : col + 1],
                        )

                # softmax over all T = 128 partitions x NT columns
                m1 = small.tile([P, 1], F32)
                nc.vector.reduce_max(m1[:], S[:], axis=mybir.AxisListType.X)
                # cross-partition max: PE transpose -> DVE reduce -> -1s matmul
                pst = psbc_pool.tile([1, P], F32, tag="misc")
                nc.tensor.transpose(pst[:], m1[:], ident[:])
                M_sb = small.tile([1, 1], F32)
                nc.vector.reduce_max(M_sb[:], pst[:], axis=mybir.AxisListType.X)
                psb = psbc_pool.tile([P, 1], F32, tag="misc")
                nc.tensor.matmul(
                    psb[:], lhsT=neg_ones_row[:], rhs=M_sb[:], start=True, stop=True
                )
                negm = small.tile([P, 1], F32)
                nc.scalar.copy(negm[:], psb[:])
                E = small.tile([P, NT], F32)
                nc.scalar.activation(
                    E[:],
                    S[:],
                    mybir.ActivationFunctionType.Exp,
                    bias=negm[:],
                    scale=1.0,
                )

                E16 = small.tile([P, NT], F16)
                nc.scalar.copy(E16[:], E[:])

                # context = sum_t E[t] * he[t, :]; Z = sum_t E[t]
                psA = psum_pool.tile([1, 512], F32)
                psB = psum_pool.tile([1, 512], F32)
                psZ = psum_pool.tile([1, 1], F32)
                for col in range(NT):
                    chunk = hets[col // 2][:, col % 2, :]
                    st = col == 0
                    sp = col == NT - 1
                    w = E16[:, col : col + 1]
                    nc.tensor.matmul(psA[:], lhsT=w, rhs=chunk[:, 0:512], start=st, stop=sp)
                    nc.tensor.matmul(psB[:], lhsT=w, rhs=chunk[:, 512:1024], start=st, stop=sp)
                    nc.tensor.matmul(psZ[:], lhsT=w, rhs=ones_col[:], start=st, stop=sp)

                # unnormalized context + Z out; host divides (keeps DVE stream
                # free of cross-engine waits)
                ob = outp.tile([1, F], F32)
                nc.scalar.copy(ob[0:1, 0:512], psA[:])
                nc.scalar.copy(ob[0:1, 512:1024], psB[:])
                nc.scalar.copy(zrow[0:1, b : b + 1], psZ[:])
                nc.sync.dma_start(out=out[b : b + 1, :], in_=ob[:])
            nc.sync.dma_start(out=zout[:], in_=zrow[:])

    nc.compile()
    return nc


def _get_nc():
    if "nc" not in _cache:
        _cache["nc"] = _build()
    return _cache["nc"]


def _run(hidden_decoder, hiddens_encoder, trace=False, tmpdir=None):
    nc = _get_nc()
    hidden_decoder = np.ascontiguousarray(hidden_decoder, dtype=np.float32)
    hiddens_encoder = np.ascontiguousarray(hiddens_encoder, dtype=np.float32)
    ident = np.eye(P, dtype=np.float32)
    in_maps = [
        {
            "he": hiddens_encoder[i * B : (i + 1) * B],
            "hd": hidden_decoder[i * B : (i + 1) * B].reshape(1, B * F),
            "ident": ident,
        }
        for i in range(N_CORES)
    ]
    res = run_bass_kernel_spmd(
        nc, in_maps, list(range(N_CORES)), trace=trace, tmpdir=tmpdir
    )
    out = np.concatenate(
        [
            res.results[i]["out"] / res.results[i]["z"].reshape(B, 1)
            for i in range(N_CORES)
        ],
        axis=0,
    ).astype(np.float32)
    return out, res


def kernel(hidden_decoder, hiddens_encoder):
    out, _ = _run(hidden_decoder, hiddens_encoder)
    return out


# revision 48
# speedup vs baseline: 1.0850x; 1.0787x over previous
"""Bass/Tile kernel for nn_DotAttention (batched dot-product attention).

  scores[b, t] = <hidden_decoder[b], hiddens_encoder[b, t]>
  a = softmax(scores, axis=t)
  context[b, f] = sum_t a[b, t] * hiddens_encoder[b, t, f]

Full shapes: hidden_decoder (64, 1024) f32, hiddens_encoder (64, 2048, 1024) f32,
output (64, 1024) f32.

Sharding: data-parallel over batch across 8 NeuronCores (8 batches/core),
no cross-device communication.

Per-core strategy (memory-bound target -- hiddens_encoder is read from HBM
exactly once, ~191 us of DMA vs the ~178 us pure-load floor):
  - he[b] loaded f32 in natural layout (t on partitions) via contiguous
    1 MiB SWDGE DMAs (SWDGE issues from the GpSimd queue, so load issue
    never stalls behind the Sync engine's semaphore waits).
  - scores: fused multiply+rowsum on VectorE (scalar_tensor_tensor with
    accum_out; full f32 precision) against a broadcast copy of
    hidden_decoder[b] (PE ones-matmul broadcast; K=1 matmul).
  - softmax max: DVE free-dim max -> PE transpose (identity) -> DVE max
    -> -1s-matmul broadcast -> ACT exp with per-partition -max bias.
    (gpsimd.partition_all_reduce and vector.tensor_tensor_reduce both
    crash this terminal's firmware -- do not use.)
  - context: PE accumulating matmuls over fp16 copies of he (cast on
    ScalarE; fp16 halves PE streaming cost and avoids the fp32 hi/lo
    double pass), with an extra ones-column matmul producing Z = sum(exp).
  - outputs are the unnormalized context plus Z; the host divides. This
    keeps VectorE's instruction stream free of cross-engine waits (a
    reciprocal on DVE head-of-line-blocked the next batch's score ops).
"""

import numpy as np

import concourse.bacc as bacc
import concourse.tile as tile
from concourse import mybir
from concourse.bass_utils import run_bass_kernel_spmd

N_CORES = 8
B_FULL = 64
B = B_FULL // N_CORES  # batches per core
T = 2048
F = 1024
P = 128
NT = T // P  # 16 t-chunks of 128
NCC = NT // 2  # 8 DMA loads of 2 chunks (1 MiB each)

F32 = mybir.dt.float32
F16 = mybir.dt.float16

_cache = {}


def _build():
    nc = bacc.Bacc("TRN2", target_bir_lowering=False, debug=False, num_devices=N_CORES)
    he = nc.dram_tensor("he", [B, T, F], F32, kind="ExternalInput").ap()
    hd = nc.dram_tensor("hd", [1, B * F], F32, kind="ExternalInput").ap()
    ident_d = nc.dram_tensor("ident", [P, P], F32, kind="ExternalInput").ap()
    out = nc.dram_tensor("out", [B, F], F32, kind="ExternalOutput").ap()
    zout = nc.dram_tensor("z", [1, B], F32, kind="ExternalOutput").ap()

    with tile.TileContext(nc) as tc:
        with (
            tc.tile_pool(name="consts", bufs=1) as consts,
            tc.tile_pool(name="hepool", bufs=10) as hepool,
            tc.tile_pool(name="he16pool", bufs=12) as he16pool,
            tc.tile_pool(name="hbc", bufs=B) as hbc,
            tc.tile_pool(name="small", bufs=3) as small,
            tc.tile_pool(name="outp", bufs=3) as outp,
            tc.tile_pool(name="psum", bufs=2, space="PSUM") as psum_pool,
            tc.tile_pool(name="psbc", bufs=2, space="PSUM") as psbc_pool,
        ):
            neg_ones_row = consts.tile([1, P], F32)  # lhsT for -max broadcast
            nc.vector.memset(neg_ones_row[:], -1.0)
            ones_col = consts.tile([P, 1], F16)  # rhs column for Z accumulation
            nc.vector.memset(ones_col[:], 1.0)
            ident = consts.tile([P, P], F32)  # identity for PE transpose
            nc.sync.dma_start(out=ident[:], in_=ident_d[:])

            ones_row = consts.tile([1, P], F32)
            nc.vector.memset(ones_row[:], 1.0)

            # broadcast hd[b] to all 128 partitions: ones(1,128)^T @ hd_row(1,F)
            hdb = []
            for b in range(B):
                hd_row = small.tile([1, F], F32, tag="hdrow")
                nc.sync.dma_start(out=hd_row[:], in_=hd[0:1, b * F : (b + 1) * F])
                t_b = hbc.tile([P, F], F32)
                for j in range(2):
                    ps = psbc_pool.tile([P, 512], F32, tag="misc")
                    nc.tensor.matmul(
                        ps[:],
                        lhsT=ones_row[:],
                        rhs=hd_row[0:1, j * 512 : (j + 1) * 512],
                        start=True,
                        stop=True,
                    )
                    nc.scalar.copy(t_b[:, j * 512 : (j + 1) * 512], ps[:])
                hdb.append(t_b)

            zrow = consts.tile([1, B], F32)

            for b in range(B):
                S = small.tile([P, NT], F32)
                dummy = small.tile([P, 1], F32)
                hets = []
                for cc in range(NCC):
                    het = hepool.tile([P, 2, F], F32)
                    nc.gpsimd.dma_start(
                        out=het[:],
                        in_=he[b, cc * 256 : (cc + 1) * 256, :].rearrange(
                            "(c p) f -> p c f", p=P
                        ),
                    )
                    # fp16 copy for the context matmul (ACT)
                    het16 = he16pool.tile([P, 2, F], F16)
                    nc.scalar.copy(het16[:], het[:])
                    hets.append(het16)
                    for c in range(2):
                        col = cc * 2 + c
                        nc.vector.scalar_tensor_tensor(
                            dummy.broadcast_to((P, F)),
                            het[:, c, :],
                            1.0,
                            hdb[b][:],
                            op0=mybir.AluOpType.mult,
                            op1=mybir.AluOpType.mult,
                            accum_out=S[:, col : col + 1],
                        )

                # softmax over all T = 128 partitions x NT columns
                m1 = small.tile([P, 1], F32)
                nc.vector.reduce_max(m1[:], S[:], axis=mybir.AxisListType.X)
                # cross-partition max: PE transpose -> DVE reduce -> -1s matmul
                pst = psbc_pool.tile([1, P], F32, tag="misc")
                nc.tensor.transpose(pst[:], m1[:], ident[:])
                M_sb = small.tile([1, 1], F32)
                nc.vector.reduce_max(M_sb[:], pst[:], axis=mybir.AxisListType.X)
                psb = psbc_pool.tile([P, 1], F32, tag="misc")
                nc.tensor.matmul(
                    psb[:], lhsT=neg_ones_row[:], rhs=M_sb[:], start=True, stop=True
                )
                negm = small.tile([P, 1], F32)
                nc.scalar.copy(negm[:], psb[:])
                E = small.tile([P, NT], F32)
                nc.scalar.activation(
                    E[:],
                    S[:],
                    mybir.ActivationFunctionType.Exp,
                    bias=negm[:],
                    scale=1.0,
                )

                E16 = small.tile([P, NT], F16)
                nc.scalar.copy(E16[:], E[:])

                # context = sum_t E[t] * he[t, :]; Z = sum_t E[t]
                psA = psum_pool.tile([1, 512], F32)
                psB = psum_pool.tile([1, 512], F32)
                psZ = psum_pool.tile([1, 1], F32)
                for col in range(NT):
                    chunk = hets[col // 2][:, col % 2, :]
                    st = col == 0
                    sp = col == NT - 1
                    w = E16[:, col : col + 1]
                    nc.tensor.matmul(psA[:], lhsT=w, rhs=chunk[:, 0:512], start=st, stop=sp)
                    nc.tensor.matmul(psB[:], lhsT=w, rhs=chunk[:, 512:1024], start=st, stop=sp)
                    nc.tensor.matmul(psZ[:], lhsT=w, rhs=ones_col[:], start=st, stop=sp)

                # unnormalized context + Z out; host divides (keeps DVE stream
                # free of cross-engine waits)
                ob = outp.tile([1, F], F32)
                nc.scalar.copy(ob[0:1, 0:512], psA[:])
                nc.scalar.copy(ob[0:1, 512:1024], psB[:])
                nc.scalar.copy(zrow[0:1, b : b + 1], psZ[:])
                nc.sync.dma_start(out=out[b : b + 1, :], in_=ob[:])
            nc.sync.dma_start(out=zout[:], in_=zrow[:])

    nc.compile()
    return nc


def _get_nc():
    if "nc" not in _cache:
        _cache["nc"] = _build()
    return _cache["nc"]


def _run(hidden_decoder, hiddens_encoder, trace=False, tmpdir=None):
    nc = _get_nc()
    hidden_decoder = np.ascontiguousarray(hidden_decoder, dtype=np.float32)
    hiddens_encoder = np.ascontiguousarray(hiddens_encoder, dtype=np.float32)
    ident = np.eye(P, dtype=np.float32)
    in_maps = [
        {
            "he": hiddens_encoder[i * B : (i + 1) * B],
            "hd": hidden_decoder[i * B : (i + 1) * B].reshape(1, B * F),
            "ident": ident,
        }
        for i in range(N_CORES)
    ]
    res = run_bass_kernel_spmd(
        nc, in_maps, list(range(N_CORES)), trace=trace, tmpdir=tmpdir
    )
    out = np.concatenate(
        [
            res.results[i]["out"] / res.results[i]["z"].reshape(B, 1)
            for i in range(N_CORES)
        ],
        axis=0,
    ).astype(np.float32)
    return out, res


def kernel(hidden_decoder, hiddens_encoder):
    out, _ = _run(hidden_decoder, hiddens_encoder)
    return out


# revision 49
# speedup vs baseline: 1.1329x; 1.0441x over previous
"""Bass/Tile kernel for nn_DotAttention (batched dot-product attention).

  scores[b, t] = <hidden_decoder[b], hiddens_encoder[b, t]>
  a = softmax(scores, axis=t)
  context[b, f] = sum_t a[b, t] * hiddens_encoder[b, t, f]

Full shapes: hidden_decoder (64, 1024) f32, hiddens_encoder (64, 2048, 1024) f32,
output (64, 1024) f32.

Sharding: data-parallel over batch across 8 NeuronCores (8 batches/core),
no cross-device communication.

Per-core strategy (memory-bound target -- hiddens_encoder is read from HBM
exactly once, ~191 us of DMA vs the ~178 us pure-load floor):
  - he[b] loaded f32 in natural layout (t on partitions) via contiguous
    1 MiB SWDGE DMAs (SWDGE issues from the GpSimd queue, so load issue
    never stalls behind the Sync engine's semaphore waits).
  - scores: fused multiply+rowsum on VectorE (scalar_tensor_tensor with
    accum_out; full f32 precision) against a broadcast copy of
    hidden_decoder[b] (PE ones-matmul broadcast; K=1 matmul).
  - softmax max: DVE free-dim max -> PE transpose (identity) -> DVE max
    -> -1s-matmul broadcast -> ACT exp with per-partition -max bias.
    (gpsimd.partition_all_reduce and vector.tensor_tensor_reduce both
    crash this terminal's firmware -- do not use.)
  - context: PE accumulating matmuls over fp16 copies of he (cast on
    ScalarE; fp16 halves PE streaming cost and avoids the fp32 hi/lo
    double pass), with an extra ones-column matmul producing Z = sum(exp).
  - outputs are the unnormalized context plus Z; the host divides. This
    keeps VectorE's instruction stream free of cross-engine waits (a
    reciprocal on DVE head-of-line-blocked the next batch's score ops).
"""

import numpy as np

import concourse.bacc as bacc
import concourse.tile as tile
from concourse import mybir
from concourse.bass_utils import run_bass_kernel_spmd

N_CORES = 8
B_FULL = 64
B = B_FULL // N_CORES  # batches per core
T = 2048
F = 1024
P = 128
NT = T // P  # 16 t-chunks of 128
NCC = NT // 2  # 8 DMA loads of 2 chunks (1 MiB each)

F32 = mybir.dt.float32
F16 = mybir.dt.float16

_cache = {}


def _build():
    nc = bacc.Bacc("TRN2", target_bir_lowering=False, debug=False, num_devices=N_CORES)
    he = nc.dram_tensor("he", [B, T, F], F32, kind="ExternalInput").ap()
    hd = nc.dram_tensor("hd", [1, B * F], F32, kind="ExternalInput").ap()
    ident_d = nc.dram_tensor("ident", [P, P], F32, kind="ExternalInput").ap()
    out = nc.dram_tensor("out", [B, F], F32, kind="ExternalOutput").ap()
    zout = nc.dram_tensor("z", [1, B], F32, kind="ExternalOutput").ap()

    with tile.TileContext(nc) as tc:
        with (
            tc.tile_pool(name="consts", bufs=1) as consts,
            tc.tile_pool(name="hepool", bufs=10) as hepool,
            tc.tile_pool(name="he16pool", bufs=12) as he16pool,
            tc.tile_pool(name="hbc", bufs=B) as hbc,
            tc.tile_pool(name="small", bufs=3) as small,
            tc.tile_pool(name="outp", bufs=3) as outp,
            tc.tile_pool(name="psum", bufs=2, space="PSUM") as psum_pool,
            tc.tile_pool(name="psbc", bufs=2, space="PSUM") as psbc_pool,
        ):
            neg_ones_row = consts.tile([1, P], F32)  # lhsT for -max broadcast
            nc.vector.memset(neg_ones_row[:], -1.0)
            ones_colf = consts.tile([P, 1], F32)  # rhs for the Z reduction matmul
            nc.vector.memset(ones_colf[:], 1.0)
            ident = consts.tile([P, P], F32)  # identity for PE transpose
            nc.sync.dma_start(out=ident[:], in_=ident_d[:])

            ones_row = consts.tile([1, P], F32)
            nc.vector.memset(ones_row[:], 1.0)

            # broadcast hd[b] to all 128 partitions: ones(1,128)^T @ hd_row(1,F)
            hdb = []
            for b in range(B):
                hd_row = small.tile([1, F], F32, tag="hdrow")
                nc.sync.dma_start(out=hd_row[:], in_=hd[0:1, b * F : (b + 1) * F])
                t_b = hbc.tile([P, F], F32)
                for j in range(2):
                    ps = psbc_pool.tile([P, 512], F32, tag="misc")
                    nc.tensor.matmul(
                        ps[:],
                        lhsT=ones_row[:],
                        rhs=hd_row[0:1, j * 512 : (j + 1) * 512],
                        start=True,
                        stop=True,
                    )
                    nc.scalar.copy(t_b[:, j * 512 : (j + 1) * 512], ps[:])
                hdb.append(t_b)

            zrow = consts.tile([1, B], F32)

            for b in range(B):
                S = small.tile([P, NT], F32)
                dummy = small.tile([P, 1], F32)
                hets = []
                for cc in range(NCC):
                    het = hepool.tile([P, 2, F], F32)
                    nc.gpsimd.dma_start(
                        out=het[:],
                        in_=he[b, cc * 256 : (cc + 1) * 256, :].rearrange(
                            "(c p) f -> p c f", p=P
                        ),
                    )
                    # fp16 copy for the context matmul (ACT)
                    het16 = he16pool.tile([P, 2, F], F16)
                    nc.scalar.copy(het16[:], het[:])
                    hets.append(het16)
                    for c in range(2):
                        col = cc * 2 + c
                        nc.vector.scalar_tensor_tensor(
                            dummy.broadcast_to((P, F)),
                            het[:, c, :],
                            1.0,
                            hdb[b][:],
                            op0=mybir.AluOpType.mult,
                            op1=mybir.AluOpType.mult,
                            accum_out=S[:, col : col + 1],
                        )

                # softmax over all T = 128 partitions x NT columns
                m1 = small.tile([P, 1], F32)
                nc.vector.reduce_max(m1[:], S[:], axis=mybir.AxisListType.X)
                # cross-partition max: PE transpose -> DVE reduce -> -1s matmul
                pst = psbc_pool.tile([1, P], F32, tag="misc")
                nc.tensor.transpose(pst[:], m1[:], ident[:])
                M_sb = small.tile([1, 1], F32)
                nc.vector.reduce_max(M_sb[:], pst[:], axis=mybir.AxisListType.X)
                psb = psbc_pool.tile([P, 1], F32, tag="misc")
                nc.tensor.matmul(
                    psb[:], lhsT=neg_ones_row[:], rhs=M_sb[:], start=True, stop=True
                )
                negm = small.tile([P, 1], F32)
                nc.scalar.copy(negm[:], psb[:])
                E = small.tile([P, NT], F32)
                z1 = small.tile([P, 1], F32)
                nc.scalar.activation(
                    E[:],
                    S[:],
                    mybir.ActivationFunctionType.Exp,
                    bias=negm[:],
                    scale=1.0,
                    accum_out=z1[:],
                )

                E16 = small.tile([P, NT], F16)
                nc.scalar.copy(E16[:], E[:])

                # context = sum_t E[t] * he[t, :]; Z = sum_t E[t]
                psA = psum_pool.tile([1, 512], F32)
                psB = psum_pool.tile([1, 512], F32)
                psZ = psum_pool.tile([1, 1], F32)
                for col in range(NT):
                    chunk = hets[col // 2][:, col % 2, :]
                    st = col == 0
                    sp = col == NT - 1
                    w = E16[:, col : col + 1]
                    nc.tensor.matmul(psA[:], lhsT=w, rhs=chunk[:, 0:512], start=st, stop=sp)
                    nc.tensor.matmul(psB[:], lhsT=w, rhs=chunk[:, 512:1024], start=st, stop=sp)
                # Z = sum_t exp = ones . z1 (z1 from the Exp accumulator)
                nc.tensor.matmul(psZ[:], lhsT=z1[:], rhs=ones_colf[:], start=True, stop=True)

                # unnormalized context + Z out; host divides (keeps DVE stream
                # free of cross-engine waits)
                ob = outp.tile([1, F], F32)
                nc.scalar.copy(ob[0:1, 0:512], psA[:])
                nc.scalar.copy(ob[0:1, 512:1024], psB[:])
                nc.scalar.copy(zrow[0:1, b : b + 1], psZ[:])
                nc.sync.dma_start(out=out[b : b + 1, :], in_=ob[:])
            nc.sync.dma_start(out=zout[:], in_=zrow[:])

    nc.compile()
    return nc


def _get_nc():
    if "nc" not in _cache:
        _cache["nc"] = _build()
    return _cache["nc"]


def _run(hidden_decoder, hiddens_encoder, trace=False, tmpdir=None):
    nc = _get_nc()
    hidden_decoder = np.ascontiguousarray(hidden_decoder, dtype=np.float32)
    hiddens_encoder = np.ascontiguousarray(hiddens_encoder, dtype=np.float32)
    ident = np.eye(P, dtype=np.float32)
    in_maps = [
        {
            "he": hiddens_encoder[i * B : (i + 1) * B],
            "hd": hidden_decoder[i * B : (i + 1) * B].reshape(1, B * F),
            "ident": ident,
        }
        for i in range(N_CORES)
    ]
    res = run_bass_kernel_spmd(
        nc, in_maps, list(range(N_CORES)), trace=trace, tmpdir=tmpdir
    )
    out = np.concatenate(
        [
            res.results[i]["out"] / res.results[i]["z"].reshape(B, 1)
            for i in range(N_CORES)
        ],
        axis=0,
    ).astype(np.float32)
    return out, res


def kernel(hidden_decoder, hiddens_encoder):
    out, _ = _run(hidden_decoder, hiddens_encoder)
    return out
